# revision 33
# baseline (speedup 1.0000x reference)
"""Bahdanau attention TRN2 kernel.

B=32, S=2048, H=U=1024. Data-parallel over batch across 8 NeuronCores
(4 batches per core), weights replicated. All matmuls in fp32r
(full-rate fp32 with ~13-bit-mantissa rounding; rounding is done inline
by the SWDGE cast DMAs).

Per core, per batch:
  projT[u, s] = tanh( (W1^T @ values^T)[u, s] + (W2^T q + b1 + b2)[u] )
  score[s]    = V^T projT          (row layout [1, S])
  attn        = softmax(score)
  context[h]  = sum_s attn[s] * values[s, h]

values enters the PE twice: transposed (h on partitions, via PE
transpose) for proj, and natural (s on partitions, re-read from HBM)
for context. bv is dropped: softmax is shift-invariant.

Scheduling notes: per-engine execution order == emission order, so the
code is laid out as a software pipeline: weights stream ut-major so the
first proj starts as soon as values tile 0 + W1[:, :128] land; each
batch's context matmuls are emitted after the NEXT batch's score phase
so the PE never waits on a softmax.
"""
import sys
sys.path.insert(0, '/opt/trn_rl_repo')
import numpy as np

B, S, H, U = 32, 2048, 1024, 1024
NCORES = 8
BL = B // NCORES          # batches per core
NK = H // 128             # h/u 128-chunks
NST = S // 512            # s-tiles of 512
MDT_NAME = "float32r"     # matmul dtype

_cache = {}


def _build(mdt_name=MDT_NAME):
    from concourse import bacc, mybir, tile

    f32 = mybir.dt.float32
    MDT = getattr(mybir.dt, mdt_name)
    AF = mybir.ActivationFunctionType
    AX = mybir.AxisListType

    nc = bacc.Bacc()
    qry = nc.declare_dram_parameter("qry", [BL, H], f32, isOutput=False)
    val = nc.declare_dram_parameter("val", [BL, S, H], f32, isOutput=False)
    W1 = nc.declare_dram_parameter("W1", [H, U], f32, isOutput=False)
    b1 = nc.declare_dram_parameter("b1", [U], f32, isOutput=False)
    W2 = nc.declare_dram_parameter("W2", [H, U], f32, isOutput=False)
    b2 = nc.declare_dram_parameter("b2", [U], f32, isOutput=False)
    Vw = nc.declare_dram_parameter("Vw", [U, 1], f32, isOutput=False)
    ctx_o = nc.declare_dram_parameter("ctx", [BL, H], f32, isOutput=True)
    att_o = nc.declare_dram_parameter("att", [BL, S], f32, isOutput=True)

    ident_d = nc.inline_tensor(np.eye(128, dtype=np.float32), name="ident")

    def quad_load(dst, b, s_base):
        """One SWDGE cast DMA: values[b, s_base:s_base+512, :] ->
        dst[128, 4*H]; column block p4 holds s rows s_base+p4*128.."""
        nc.gpsimd.dma_start(
            dst[:].rearrange("p (p4 h) -> p p4 h", p4=4),
            val[b, s_base:s_base + 512, :].rearrange(
                "(p4 p) h -> p p4 h", p=128))

    def wu_load(dst, Wt, ut):
        """dst[128, NK*128] <- W[:, ut*128:(ut+1)*128] with h-chunk k at
        column block k: dst[p, k*128+u] = W[k*128+p, ut*128+u]."""
        nc.gpsimd.dma_start(
            dst[:].rearrange("p (k u) -> p k u", k=NK),
            Wt[:, ut * 128:(ut + 1) * 128].rearrange("(k p) u -> p k u", p=128))

    with tile.TileContext(nc) as tc:
        with tc.tile_pool(name="const", bufs=1) as cpool, \
             tc.tile_pool(name="nat", bufs=2) as natp, \
             tc.tile_pool(name="valT", bufs=20) as valtp, \
             tc.tile_pool(name="tanh", bufs=3) as tanhp, \
             tc.tile_pool(name="scorep", bufs=2) as scorep, \
             tc.tile_pool(name="rows", bufs=2) as rowp, \
             tc.tile_pool(name="wps", bufs=4, space="PSUM") as wps, \
             tc.tile_pool(name="sps", bufs=2, space="PSUM") as sps, \
             tc.tile_pool(name="cps", bufs=2, space="PSUM") as cps:

            # ---- DMA emission order is load order: get the critical path
            # (ident, first two value quads, W1/W2 col 0) in first ----
            ident = cpool.tile([128, 128], MDT, tag="ident")
            nc.gpsimd.dma_start(ident[:], ident_d[:])
            first_quads = {}
            for st in range(2):
                q_t = natp.tile([128, 4 * H], MDT, tag="nat")
                quad_load(q_t, 0, st * 512)
                first_quads[st] = q_t
            w1u, w2u = [], []
            for ut in range(NK):
                w1_t = cpool.tile([128, NK * 128], MDT, tag=f"w1_{ut}")
                w1u.append(w1_t)
                w2_t = cpool.tile([128, NK * 128], MDT, tag=f"w2_{ut}")
                w2u.append(w2_t)
            wu_load(w1u[0], W1, 0)
            wu_load(w2u[0], W2, 0)
            # small constants
            v_col = cpool.tile([128, NK], MDT, tag="v_col")
            nc.gpsimd.dma_start(v_col[:], Vw[:, 0].rearrange("(k p) -> p k", p=128))
            q_col = cpool.tile([128, NK * BL], MDT, tag="q_col")
            for b in range(BL):
                nc.gpsimd.dma_start(
                    q_col[:].rearrange("p (k b) -> p k b", b=BL)[:, :, b],
                    qry[b, :].rearrange("(k p) -> p k", p=128))
            b12 = cpool.tile([128, NK], f32, tag="b12")
            b2c = cpool.tile([128, NK], f32, tag="b2c")
            nc.sync.dma_start(b12[:], b1[:].rearrange("(k p) -> p k", p=128))
            nc.sync.dma_start(b2c[:], b2[:].rearrange("(k p) -> p k", p=128))
            nc.vector.tensor_add(b12[:], b12[:], b2c[:])
            ident1 = cpool.tile([1, 1], f32, tag="ident1")
            nc.vector.memset(ident1[:], 1.0)
            # rest of the weights stream in behind the first compute
            for ut in range(1, NK):
                wu_load(w1u[ut], W1, ut)
                wu_load(w2u[ut], W2, ut)

            qbT = []
            for u in range(NK):
                qb_t = cpool.tile([128, BL], f32, tag=f"qb_{u}")
                qbT.append(qb_t)

            kept_quads = {}

            def prep_scores(b, after_first_pair=None):
                """loads + transposes + proj/tanh/score for one batch."""
                valT = {}
                score_sb = scorep.tile([1, S], f32, tag="score")

                def load_and_transpose_pair(stp):
                    for st in (stp * 2, stp * 2 + 1):
                        if (b, st) in ((0, 0), (0, 1)):
                            quad = first_quads.pop(st)
                        else:
                            quad = natp.tile([128, 4 * H], MDT, tag="nat")
                            quad_load(quad, b, st * 512)
                        if b == BL - 1 and st >= NST - 2:
                            # nothing reuses the nat pool after the final
                            # prep: these tiles stay resident for its context
                            kept_quads[st] = quad
                        for hc in range(NK):
                            tp = wps.tile([128, 512], MDT, tag="wp")
                            for p4 in range(4):
                                nc.tensor.transpose(
                                    tp[:, p4 * 128:(p4 + 1) * 128],
                                    quad[:, p4 * H + hc * 128:
                                         p4 * H + (hc + 1) * 128],
                                    ident[:])
                            vt = valtp.tile([128, 512], MDT, tag="vt")
                            if hc % 2 == 0:
                                nc.vector.tensor_copy(vt[:], tp[:])
                            else:
                                nc.scalar.copy(vt[:], tp[:])
                            valT[(st, hc)] = vt

                for stp in range(NST // 2):
                    load_and_transpose_pair(stp)
                    if stp == 0 and after_first_pair is not None:
                        after_first_pair()
                    sc_ps = []
                    for _i in range(2):
                        sc_t = sps.tile([1, 512], f32, tag="sc")
                        sc_ps.append(sc_t)
                    for ut in range(NK):
                        pps = []
                        for _i in range(2):
                            pp_t = wps.tile([128, 512], f32, tag="wp")
                            pps.append(pp_t)
                        for k in range(NK):
                            for st2 in range(2):
                                nc.tensor.matmul(
                                    pps[st2][:],
                                    w1u[ut][:, k * 128:(k + 1) * 128],
                                    valT[(stp * 2 + st2, k)][:],
                                    start=(k == 0), stop=(k == NK - 1))
                        if b == 0 and stp == 0:
                            # qbiasT[ut] = W2^T q + b1 + b2, squeezed into
                            # the pipeline just before its first use
                            qb_ps = wps.tile([128, BL], f32, tag="wp")
                            for k in range(NK):
                                nc.tensor.matmul(
                                    qb_ps[:], w2u[ut][:, k * 128:(k + 1) * 128],
                                    q_col[:, k * BL:(k + 1) * BL],
                                    start=(k == 0), stop=(k == NK - 1))
                            nc.vector.tensor_scalar_add(
                                qbT[ut][:], qb_ps[:], b12[:, ut:ut + 1])
                        for st2 in range(2):
                            th = tanhp.tile([128, 512], MDT, tag="th")
                            nc.scalar.activation(
                                th[:], pps[st2][:], AF.Tanh,
                                bias=qbT[ut][:, b:b + 1])
                            nc.tensor.matmul(
                                sc_ps[st2][:], v_col[:, ut:ut + 1], th[:],
                                start=(ut == 0), stop=(ut == NK - 1))
                    for st2 in range(2):
                        st = stp * 2 + st2
                        nc.vector.tensor_copy(
                            score_sb[:, st * 512:(st + 1) * 512],
                            sc_ps[st2][:])
                return score_sb

            def softmax_acol(b, score_sb):
                """softmax (in place on the score row) + column relayout."""
                m = rowp.tile([1, 1], f32, tag="m")
                nc.vector.reduce_max(m[:], score_sb[:], axis=AX.X)
                negm = rowp.tile([1, 1], f32, tag="negm")
                nc.vector.tensor_scalar_mul(negm[:], m[:], -1.0)
                esum = rowp.tile([1, 1], f32, tag="esum")
                nc.scalar.activation(score_sb[:], score_sb[:], AF.Exp,
                                     bias=negm[:], accum_out=esum[:])
                inv = rowp.tile([1, 1], f32, tag="inv")
                nc.vector.reciprocal(inv[:], esum[:])
                nc.vector.tensor_scalar_mul(score_sb[:], score_sb[:], inv[:])
                nc.sync.dma_start(att_o[b, :].rearrange("(o f) -> o f", o=1),
                                  score_sb[:])
                acps = sps.tile([128, S // 128], f32, tag="sc")
                for c in range(S // 128):
                    nc.tensor.matmul(
                        acps[:, c:c + 1],
                        score_sb[:, c * 128:(c + 1) * 128], ident1[:],
                        start=True, stop=True)
                acol = rowp.tile([128, S // 128], MDT, tag="acol")
                nc.vector.tensor_copy(acol[:], acps[:])
                return acol

            def cq_load(cnatp, b, pair):
                cq = cnatp.tile([128, 2 * H], MDT, tag="cnat")
                nc.gpsimd.dma_start(
                    cq[:].rearrange("p (p2 h) -> p p2 h", p2=2),
                    val[b, pair * 256:(pair + 1) * 256, :].rearrange(
                        "(p2 p) h -> p p2 h", p=128))
                return cq

            def ctx_batch(b, acol, cnatp, preloaded=()):
                """context = attn^T values, restreaming values from HBM."""
                crow = rowp.tile([1, H], f32, tag="crow")
                cp = []
                for _i in range(2):
                    cp_t = cps.tile([1, 512], f32, tag="cp")
                    cp.append(cp_t)
                cq = None
                preloaded = list(preloaded)
                resident = kept_quads if b == BL - 1 else {}
                for sc in range(S // 128):
                    st = sc // 4
                    if st in resident:
                        src = resident[st]
                        base = (sc % 4) * H
                    else:
                        if sc % 2 == 0:
                            if preloaded:
                                cq = preloaded.pop(0)
                            else:
                                cq = cq_load(cnatp, b, sc // 2)
                        src = cq
                        base = (sc % 2) * H
                    for half in range(2):
                        nc.tensor.matmul(
                            cp[half][:], acol[:, sc:sc + 1],
                            src[:, base + half * 512:base + half * 512 + 512],
                            start=(sc == 0), stop=(sc == S // 128 - 1))
                for half in range(2):
                    nc.vector.tensor_copy(
                        crow[:, half * 512:(half + 1) * 512], cp[half][:])
                nc.sync.dma_start(ctx_o[b, :].rearrange("(o f) -> o f", o=1),
                                  crow[:])

            # pipeline: A(b)=prep_scores, B(b)=softmax_acol, C(b)=ctx
            # order: A0 A1 B0 C0 A2 B1 C1 A3 B2 C2 B3 C3
            scores, acols = {}, {}
            with tc.tile_pool(name="cpre", bufs=4) as cnatp:
                scores[0] = prep_scores(0)
                for b in range(1, BL):
                    pre = []

                    def inject(bb=b - 1, dst=pre):
                        for p in range(4):
                            dst.append(cq_load(cnatp, bb, p))

                    scores[b] = prep_scores(b, after_first_pair=inject)
                    acols[b - 1] = softmax_acol(b - 1, scores.pop(b - 1))
                    ctx_batch(b - 1, acols.pop(b - 1), cnatp, pre)
                # prefetch part of the last batch's context quads so its
                # context matmuls aren't DMA-paced at the tail
                pre3 = [cq_load(cnatp, BL - 1, p) for p in range(4)]
                acols[BL - 1] = softmax_acol(BL - 1, scores.pop(BL - 1))
                ctx_batch(BL - 1, acols.pop(BL - 1), cnatp, pre3)

    nc.finalize()
    return nc


def _get_nc():
    key = MDT_NAME
    if key not in _cache:
        _cache[key] = _build(key)
    return _cache[key]


def kernel(query, values, W1, b1, W2, b2, V, bv):
    from concourse.bass_utils import run_bass_kernel_spmd

    query = np.ascontiguousarray(np.asarray(query, dtype=np.float32))
    values = np.ascontiguousarray(np.asarray(values, dtype=np.float32))
    W1 = np.ascontiguousarray(np.asarray(W1, dtype=np.float32))
    b1 = np.ascontiguousarray(np.asarray(b1, dtype=np.float32))
    W2 = np.ascontiguousarray(np.asarray(W2, dtype=np.float32))
    b2 = np.ascontiguousarray(np.asarray(b2, dtype=np.float32))
    V = np.ascontiguousarray(np.asarray(V, dtype=np.float32))
    bv = np.asarray(bv, dtype=np.float32)  # softmax-invariant; unused

    nc = _get_nc()
    in_maps = []
    for c in range(NCORES):
        lo = c * BL
        in_maps.append({
            "qry": query[lo:lo + BL],
            "val": values[lo:lo + BL],
            "W1": W1, "b1": b1, "W2": W2, "b2": b2, "Vw": V,
        })
    res = run_bass_kernel_spmd(nc, in_maps, list(range(NCORES)))
    ctx = np.concatenate([res.results[c]["ctx"] for c in range(NCORES)], axis=0)
    att = np.concatenate([res.results[c]["att"] for c in range(NCORES)], axis=0)
    return ctx, att.reshape(B, S, 1)


# revision 34
# speedup vs baseline: 1.0324x; 1.0324x over previous
"""Bahdanau attention TRN2 kernel.

B=32, S=2048, H=U=1024. Data-parallel over batch across 8 NeuronCores
(4 batches per core), weights replicated. All matmuls in fp32r
(full-rate fp32 with ~13-bit-mantissa rounding; rounding is done inline
by the SWDGE cast DMAs).

Per core, per batch:
  projT[u, s] = tanh( (W1^T @ values^T)[u, s] + (W2^T q + b1 + b2)[u] )
  score[s]    = V^T projT          (row layout [1, S])
  attn        = softmax(score)
  context[h]  = sum_s attn[s] * values[s, h]

values enters the PE twice: transposed (h on partitions, via PE
transpose) for proj, and natural (s on partitions, re-read from HBM)
for context. bv is dropped: softmax is shift-invariant.

Scheduling notes: per-engine execution order == emission order, so the
code is laid out as a software pipeline: weights stream ut-major so the
first proj starts as soon as values tile 0 + W1[:, :128] land; each
batch's context matmuls are emitted after the NEXT batch's score phase
so the PE never waits on a softmax.
"""
import sys
sys.path.insert(0, '/opt/trn_rl_repo')
import numpy as np

B, S, H, U = 32, 2048, 1024, 1024
NCORES = 8
BL = B // NCORES          # batches per core
NK = H // 128             # h/u 128-chunks
NST = S // 512            # s-tiles of 512
MDT_NAME = "float32r"     # matmul dtype

_cache = {}


def _build(mdt_name=MDT_NAME):
    from concourse import bacc, mybir, tile

    f32 = mybir.dt.float32
    MDT = getattr(mybir.dt, mdt_name)
    AF = mybir.ActivationFunctionType
    AX = mybir.AxisListType

    nc = bacc.Bacc()
    qry = nc.declare_dram_parameter("qry", [BL, H], f32, isOutput=False)
    val = nc.declare_dram_parameter("val", [BL, S, H], f32, isOutput=False)
    W1 = nc.declare_dram_parameter("W1", [H, U], f32, isOutput=False)
    b1 = nc.declare_dram_parameter("b1", [U], f32, isOutput=False)
    W2 = nc.declare_dram_parameter("W2", [H, U], f32, isOutput=False)
    b2 = nc.declare_dram_parameter("b2", [U], f32, isOutput=False)
    Vw = nc.declare_dram_parameter("Vw", [U, 1], f32, isOutput=False)
    ctx_o = nc.declare_dram_parameter("ctx", [BL, H], f32, isOutput=True)
    att_o = nc.declare_dram_parameter("att", [BL, S], f32, isOutput=True)

    ident_d = nc.inline_tensor(np.eye(128, dtype=np.float32), name="ident")

    def quad_load(dst, b, s_base):
        """One SWDGE cast DMA: values[b, s_base:s_base+512, :] ->
        dst[128, 4*H]; column block p4 holds s rows s_base+p4*128.."""
        nc.gpsimd.dma_start(
            dst[:].rearrange("p (p4 h) -> p p4 h", p4=4),
            val[b, s_base:s_base + 512, :].rearrange(
                "(p4 p) h -> p p4 h", p=128))

    def wu_load(dst, Wt, ut):
        """dst[128, NK*128] <- W[:, ut*128:(ut+1)*128] with h-chunk k at
        column block k: dst[p, k*128+u] = W[k*128+p, ut*128+u]."""
        nc.gpsimd.dma_start(
            dst[:].rearrange("p (k u) -> p k u", k=NK),
            Wt[:, ut * 128:(ut + 1) * 128].rearrange("(k p) u -> p k u", p=128))

    with tile.TileContext(nc) as tc:
        with tc.tile_pool(name="const", bufs=1) as cpool, \
             tc.tile_pool(name="nat", bufs=2) as natp, \
             tc.tile_pool(name="valT", bufs=20) as valtp, \
             tc.tile_pool(name="tanh", bufs=3) as tanhp, \
             tc.tile_pool(name="scorep", bufs=2) as scorep, \
             tc.tile_pool(name="rows", bufs=2) as rowp, \
             tc.tile_pool(name="wps", bufs=4, space="PSUM") as wps, \
             tc.tile_pool(name="sps", bufs=2, space="PSUM") as sps, \
             tc.tile_pool(name="cps", bufs=2, space="PSUM") as cps:

            # ---- DMA emission order is load order: get the critical path
            # (ident, first two value quads, W1/W2 col 0) in first ----
            ident = cpool.tile([128, 128], MDT, tag="ident")
            nc.gpsimd.dma_start(ident[:], ident_d[:])
            first_quads = {}
            for st in range(2):
                q_t = natp.tile([128, 4 * H], MDT, tag="nat")
                quad_load(q_t, 0, st * 512)
                first_quads[st] = q_t
            w2ctx = tc.tile_pool(name="w2p", bufs=1)
            w2pool = w2ctx.__enter__()
            w1u, w2u = [], []
            for ut in range(NK):
                w1_t = cpool.tile([128, NK * 128], MDT, tag=f"w1_{ut}")
                w1u.append(w1_t)
                w2_t = w2pool.tile([128, NK * 128], MDT, tag=f"w2_{ut}")
                w2u.append(w2_t)
            wu_load(w1u[0], W1, 0)
            wu_load(w2u[0], W2, 0)
            # small constants
            v_col = cpool.tile([128, NK], MDT, tag="v_col")
            nc.gpsimd.dma_start(v_col[:], Vw[:, 0].rearrange("(k p) -> p k", p=128))
            q_col = cpool.tile([128, NK * BL], MDT, tag="q_col")
            for b in range(BL):
                nc.gpsimd.dma_start(
                    q_col[:].rearrange("p (k b) -> p k b", b=BL)[:, :, b],
                    qry[b, :].rearrange("(k p) -> p k", p=128))
            b12 = cpool.tile([128, NK], f32, tag="b12")
            b2c = cpool.tile([128, NK], f32, tag="b2c")
            nc.sync.dma_start(b12[:], b1[:].rearrange("(k p) -> p k", p=128))
            nc.sync.dma_start(b2c[:], b2[:].rearrange("(k p) -> p k", p=128))
            nc.vector.tensor_add(b12[:], b12[:], b2c[:])
            ident1 = cpool.tile([1, 1], f32, tag="ident1")
            nc.vector.memset(ident1[:], 1.0)
            # rest of the weights stream in behind the first compute
            for ut in range(1, NK):
                wu_load(w1u[ut], W1, ut)
                wu_load(w2u[ut], W2, ut)

            qbT = []
            for u in range(NK):
                qb_t = cpool.tile([128, BL], f32, tag=f"qb_{u}")
                qbT.append(qb_t)

            kept_quads = {}
            kept_quads2 = {}
            pool_ref = {}

            def prep_scores(b, after_first_pair=None):
                """loads + transposes + proj/tanh/score for one batch."""
                valT = {}
                score_sb = scorep.tile([1, S], f32, tag="score")

                def load_and_transpose_pair(stp):
                    for st in (stp * 2, stp * 2 + 1):
                        if (b, st) in ((0, 0), (0, 1)):
                            quad = first_quads.pop(st)
                        elif b == BL - 2 and st >= NST - 2:
                            # batch BL-2's last pair stays resident (in the
                            # SBUF released by the W2 pool) so its context -
                            # which runs while batch BL-1 softmaxes - halves
                            # its HBM re-read
                            quad = pool_ref["keep"].tile(
                                [128, 4 * H], MDT, tag="keep")
                            quad_load(quad, b, st * 512)
                            kept_quads2[st] = quad
                        else:
                            quad = natp.tile([128, 4 * H], MDT, tag="nat")
                            quad_load(quad, b, st * 512)
                        if b == BL - 1 and st >= NST - 2:
                            # nothing reuses the nat pool after the final
                            # prep: these tiles stay resident for its context
                            kept_quads[st] = quad
                        for hc in range(NK):
                            tp = wps.tile([128, 512], MDT, tag="wp")
                            for p4 in range(4):
                                nc.tensor.transpose(
                                    tp[:, p4 * 128:(p4 + 1) * 128],
                                    quad[:, p4 * H + hc * 128:
                                         p4 * H + (hc + 1) * 128],
                                    ident[:])
                            vt = valtp.tile([128, 512], MDT, tag="vt")
                            if hc % 2 == 0:
                                nc.vector.tensor_copy(vt[:], tp[:])
                            else:
                                nc.scalar.copy(vt[:], tp[:])
                            valT[(st, hc)] = vt

                for stp in range(NST // 2):
                    load_and_transpose_pair(stp)
                    if stp == 0 and after_first_pair is not None:
                        after_first_pair()
                    sc_ps = []
                    for _i in range(2):
                        sc_t = sps.tile([1, 512], f32, tag="sc")
                        sc_ps.append(sc_t)
                    for ut in range(NK):
                        pps = []
                        for _i in range(2):
                            pp_t = wps.tile([128, 512], f32, tag="wp")
                            pps.append(pp_t)
                        for k in range(NK):
                            for st2 in range(2):
                                nc.tensor.matmul(
                                    pps[st2][:],
                                    w1u[ut][:, k * 128:(k + 1) * 128],
                                    valT[(stp * 2 + st2, k)][:],
                                    start=(k == 0), stop=(k == NK - 1))
                        if b == 0 and stp == 0:
                            # qbiasT[ut] = W2^T q + b1 + b2, squeezed into
                            # the pipeline just before its first use
                            qb_ps = wps.tile([128, BL], f32, tag="wp")
                            for k in range(NK):
                                nc.tensor.matmul(
                                    qb_ps[:], w2u[ut][:, k * 128:(k + 1) * 128],
                                    q_col[:, k * BL:(k + 1) * BL],
                                    start=(k == 0), stop=(k == NK - 1))
                            nc.vector.tensor_scalar_add(
                                qbT[ut][:], qb_ps[:], b12[:, ut:ut + 1])
                        for st2 in range(2):
                            th = tanhp.tile([128, 512], MDT, tag="th")
                            nc.scalar.activation(
                                th[:], pps[st2][:], AF.Tanh,
                                bias=qbT[ut][:, b:b + 1])
                            nc.tensor.matmul(
                                sc_ps[st2][:], v_col[:, ut:ut + 1], th[:],
                                start=(ut == 0), stop=(ut == NK - 1))
                    for st2 in range(2):
                        st = stp * 2 + st2
                        nc.vector.tensor_copy(
                            score_sb[:, st * 512:(st + 1) * 512],
                            sc_ps[st2][:])
                return score_sb

            def softmax_acol(b, score_sb):
                """softmax (in place on the score row) + column relayout."""
                m = rowp.tile([1, 1], f32, tag="m")
                nc.vector.reduce_max(m[:], score_sb[:], axis=AX.X)
                negm = rowp.tile([1, 1], f32, tag="negm")
                nc.vector.tensor_scalar_mul(negm[:], m[:], -1.0)
                esum = rowp.tile([1, 1], f32, tag="esum")
                nc.scalar.activation(score_sb[:], score_sb[:], AF.Exp,
                                     bias=negm[:], accum_out=esum[:])
                inv = rowp.tile([1, 1], f32, tag="inv")
                nc.vector.reciprocal(inv[:], esum[:])
                nc.vector.tensor_scalar_mul(score_sb[:], score_sb[:], inv[:])
                nc.sync.dma_start(att_o[b, :].rearrange("(o f) -> o f", o=1),
                                  score_sb[:])
                acps = sps.tile([128, S // 128], f32, tag="sc")
                for c in range(S // 128):
                    nc.tensor.matmul(
                        acps[:, c:c + 1],
                        score_sb[:, c * 128:(c + 1) * 128], ident1[:],
                        start=True, stop=True)
                acol = rowp.tile([128, S // 128], MDT, tag="acol")
                nc.vector.tensor_copy(acol[:], acps[:])
                return acol

            def cq_load(cnatp, b, pair):
                cq = cnatp.tile([128, 2 * H], MDT, tag="cnat")
                nc.gpsimd.dma_start(
                    cq[:].rearrange("p (p2 h) -> p p2 h", p2=2),
                    val[b, pair * 256:(pair + 1) * 256, :].rearrange(
                        "(p2 p) h -> p p2 h", p=128))
                return cq

            def ctx_batch(b, acol, cnatp, preloaded=()):
                """context = attn^T values, restreaming values from HBM."""
                crow = rowp.tile([1, H], f32, tag="crow")
                cp = []
                for _i in range(2):
                    cp_t = cps.tile([1, 512], f32, tag="cp")
                    cp.append(cp_t)
                cq = None
                preloaded = list(preloaded)
                resident = (kept_quads if b == BL - 1
                            else kept_quads2 if b == BL - 2 else {})
                for sc in range(S // 128):
                    st = sc // 4
                    if st in resident:
                        src = resident[st]
                        base = (sc % 4) * H
                    else:
                        if sc % 2 == 0:
                            if preloaded:
                                cq = preloaded.pop(0)
                            else:
                                cq = cq_load(cnatp, b, sc // 2)
                        src = cq
                        base = (sc % 2) * H
                    for half in range(2):
                        nc.tensor.matmul(
                            cp[half][:], acol[:, sc:sc + 1],
                            src[:, base + half * 512:base + half * 512 + 512],
                            start=(sc == 0), stop=(sc == S // 128 - 1))
                for half in range(2):
                    nc.vector.tensor_copy(
                        crow[:, half * 512:(half + 1) * 512], cp[half][:])
                nc.sync.dma_start(ctx_o[b, :].rearrange("(o f) -> o f", o=1),
                                  crow[:])

            # pipeline: A(b)=prep_scores, B(b)=softmax_acol, C(b)=ctx
            # order: A0 A1 B0 C0 A2 B1 C1 A3 B2 C2 B3 C3
            scores, acols = {}, {}
            scores[0] = prep_scores(0)
            w2ctx.__exit__(None, None, None)
            with tc.tile_pool(name="cpre", bufs=4) as cnatp, \
                 tc.tile_pool(name="keep", bufs=2) as keepp:
                pool_ref["keep"] = keepp
                for b in range(1, BL):
                    pre = []

                    def inject(bb=b - 1, dst=pre):
                        for p in range(4):
                            dst.append(cq_load(cnatp, bb, p))

                    scores[b] = prep_scores(b, after_first_pair=inject)
                    acols[b - 1] = softmax_acol(b - 1, scores.pop(b - 1))
                    ctx_batch(b - 1, acols.pop(b - 1), cnatp, pre)
                # prefetch part of the last batch's context quads so its
                # context matmuls aren't DMA-paced at the tail
                pre3 = [cq_load(cnatp, BL - 1, p) for p in range(4)]
                acols[BL - 1] = softmax_acol(BL - 1, scores.pop(BL - 1))
                ctx_batch(BL - 1, acols.pop(BL - 1), cnatp, pre3)

    nc.finalize()
    return nc


def _get_nc():
    key = MDT_NAME
    if key not in _cache:
        _cache[key] = _build(key)
    return _cache[key]


def kernel(query, values, W1, b1, W2, b2, V, bv):
    from concourse.bass_utils import run_bass_kernel_spmd

    query = np.ascontiguousarray(np.asarray(query, dtype=np.float32))
    values = np.ascontiguousarray(np.asarray(values, dtype=np.float32))
    W1 = np.ascontiguousarray(np.asarray(W1, dtype=np.float32))
    b1 = np.ascontiguousarray(np.asarray(b1, dtype=np.float32))
    W2 = np.ascontiguousarray(np.asarray(W2, dtype=np.float32))
    b2 = np.ascontiguousarray(np.asarray(b2, dtype=np.float32))
    V = np.ascontiguousarray(np.asarray(V, dtype=np.float32))
    bv = np.asarray(bv, dtype=np.float32)  # softmax-invariant; unused

    nc = _get_nc()
    in_maps = []
    for c in range(NCORES):
        lo = c * BL
        in_maps.append({
            "qry": query[lo:lo + BL],
            "val": values[lo:lo + BL],
            "W1": W1, "b1": b1, "W2": W2, "b2": b2, "Vw": V,
        })
    res = run_bass_kernel_spmd(nc, in_maps, list(range(NCORES)))
    ctx = np.concatenate([res.results[c]["ctx"] for c in range(NCORES)], axis=0)
    att = np.concatenate([res.results[c]["att"] for c in range(NCORES)], axis=0)
    return ctx, att.reshape(B, S, 1)


# revision 37
# speedup vs baseline: 1.0458x; 1.0130x over previous
"""Bahdanau attention TRN2 kernel.

B=32, S=2048, H=U=1024. Data-parallel over batch across 8 NeuronCores
(4 batches per core), weights replicated. All matmuls in fp32r
(full-rate fp32 with ~13-bit-mantissa rounding; rounding is done inline
by the SWDGE cast DMAs).

Per core, per batch:
  projT[u, s] = tanh( (W1^T @ values^T)[u, s] + (W2^T q + b1 + b2)[u] )
  score[s]    = V^T projT          (row layout [1, S])
  attn        = softmax(score)
  context[h]  = sum_s attn[s] * values[s, h]

values enters the PE twice: transposed (h on partitions, via PE
transpose) for proj, and natural (s on partitions, re-read from HBM)
for context. bv is dropped: softmax is shift-invariant.

Scheduling notes: per-engine execution order == emission order, so the
code is laid out as a software pipeline: weights stream ut-major so the
first proj starts as soon as values tile 0 + W1[:, :128] land; each
batch's context matmuls are emitted after the NEXT batch's score phase
so the PE never waits on a softmax.
"""
import sys
sys.path.insert(0, '/opt/trn_rl_repo')
import numpy as np

B, S, H, U = 32, 2048, 1024, 1024
NCORES = 8
BL = B // NCORES          # batches per core
NK = H // 128             # h/u 128-chunks
NST = S // 512            # s-tiles of 512
MDT_NAME = "float32r"     # matmul dtype

_cache = {}


def _build(mdt_name=MDT_NAME):
    from concourse import bacc, mybir, tile

    f32 = mybir.dt.float32
    MDT = getattr(mybir.dt, mdt_name)
    AF = mybir.ActivationFunctionType
    AX = mybir.AxisListType

    nc = bacc.Bacc()
    qry = nc.declare_dram_parameter("qry", [BL, H], f32, isOutput=False)
    val = nc.declare_dram_parameter("val", [BL, S, H], f32, isOutput=False)
    W1 = nc.declare_dram_parameter("W1", [H, U], f32, isOutput=False)
    b1 = nc.declare_dram_parameter("b1", [U], f32, isOutput=False)
    W2 = nc.declare_dram_parameter("W2", [H, U], f32, isOutput=False)
    b2 = nc.declare_dram_parameter("b2", [U], f32, isOutput=False)
    Vw = nc.declare_dram_parameter("Vw", [U, 1], f32, isOutput=False)
    ctx_o = nc.declare_dram_parameter("ctx", [BL, H], f32, isOutput=True)
    att_o = nc.declare_dram_parameter("att", [BL, S], f32, isOutput=True)

    ident_d = nc.inline_tensor(np.eye(128, dtype=np.float32), name="ident")

    def quad_load(dst, b, s_base):
        """One SWDGE cast DMA: values[b, s_base:s_base+512, :] ->
        dst[128, 4*H]; column block p4 holds s rows s_base+p4*128.."""
        nc.gpsimd.dma_start(
            dst[:].rearrange("p (p4 h) -> p p4 h", p4=4),
            val[b, s_base:s_base + 512, :].rearrange(
                "(p4 p) h -> p p4 h", p=128))

    def wu_load(dst, Wt, ut):
        """dst[128, NK*128] <- W[:, ut*128:(ut+1)*128] with h-chunk k at
        column block k: dst[p, k*128+u] = W[k*128+p, ut*128+u]."""
        nc.gpsimd.dma_start(
            dst[:].rearrange("p (k u) -> p k u", k=NK),
            Wt[:, ut * 128:(ut + 1) * 128].rearrange("(k p) u -> p k u", p=128))

    with tile.TileContext(nc) as tc:
        with tc.tile_pool(name="const", bufs=1) as cpool, \
             tc.tile_pool(name="nat", bufs=2) as natp, \
             tc.tile_pool(name="valT", bufs=20) as valtp, \
             tc.tile_pool(name="tanh", bufs=3) as tanhp, \
             tc.tile_pool(name="scorep", bufs=2) as scorep, \
             tc.tile_pool(name="rows", bufs=2) as rowp, \
             tc.tile_pool(name="wps", bufs=4, space="PSUM") as wps, \
             tc.tile_pool(name="sps", bufs=2, space="PSUM") as sps, \
             tc.tile_pool(name="cps", bufs=2, space="PSUM") as cps:

            # ---- DMA emission order is load order: get the critical path
            # (ident, first two value quads, W1/W2 col 0) in first ----
            ident = cpool.tile([128, 128], MDT, tag="ident")
            nc.gpsimd.dma_start(ident[:], ident_d[:])
            first_quads = {}
            for st in range(2):
                q_t = natp.tile([128, 4 * H], MDT, tag="nat")
                quad_load(q_t, 0, st * 512)
                first_quads[st] = q_t
            w2ctx = tc.tile_pool(name="w2p", bufs=1)
            w2pool = w2ctx.__enter__()
            w1u, w2u = [], []
            for ut in range(NK):
                w1_t = cpool.tile([128, NK * 128], MDT, tag=f"w1_{ut}")
                w1u.append(w1_t)
                w2_t = w2pool.tile([128, NK * 128], MDT, tag=f"w2_{ut}")
                w2u.append(w2_t)
            wu_load(w1u[0], W1, 0)
            wu_load(w2u[0], W2, 0)
            # small constants
            v_col = cpool.tile([128, NK], MDT, tag="v_col")
            nc.gpsimd.dma_start(v_col[:], Vw[:, 0].rearrange("(k p) -> p k", p=128))
            q_col = cpool.tile([128, NK * BL], MDT, tag="q_col")
            for b in range(BL):
                nc.gpsimd.dma_start(
                    q_col[:].rearrange("p (k b) -> p k b", b=BL)[:, :, b],
                    qry[b, :].rearrange("(k p) -> p k", p=128))
            b12 = cpool.tile([128, NK], f32, tag="b12")
            b2c = cpool.tile([128, NK], f32, tag="b2c")
            nc.sync.dma_start(b12[:], b1[:].rearrange("(k p) -> p k", p=128))
            nc.sync.dma_start(b2c[:], b2[:].rearrange("(k p) -> p k", p=128))
            nc.vector.tensor_add(b12[:], b12[:], b2c[:])
            ident1 = cpool.tile([1, 1], f32, tag="ident1")
            nc.vector.memset(ident1[:], 1.0)
            # rest of the weights stream in behind the first compute
            for ut in range(1, NK):
                wu_load(w1u[ut], W1, ut)
                wu_load(w2u[ut], W2, ut)

            qbT = []
            for u in range(NK):
                qb_t = cpool.tile([128, BL], f32, tag=f"qb_{u}")
                qbT.append(qb_t)

            kept_quads = {}
            kept_quads2 = {}
            pool_ref = {}

            def prep_scores(b, after_first_pair=None):
                """loads + transposes + proj/tanh/score for one batch."""
                valT = {}
                score_sb = scorep.tile([1, S], f32, tag="score")

                def load_and_transpose_pair(stp):
                    for st in (stp * 2, stp * 2 + 1):
                        if (b, st) in ((0, 0), (0, 1)):
                            quad = first_quads.pop(st)
                        elif b == BL - 2 and st >= NST - 2:
                            # batch BL-2's last pair stays resident (in the
                            # SBUF released by the W2 pool) so its context -
                            # which runs while batch BL-1 softmaxes - halves
                            # its HBM re-read
                            quad = pool_ref["keep"].tile(
                                [128, 4 * H], MDT, tag="keep")
                            quad_load(quad, b, st * 512)
                            kept_quads2[st] = quad
                        else:
                            quad = natp.tile([128, 4 * H], MDT, tag="nat")
                            quad_load(quad, b, st * 512)
                        if b == BL - 1 and st >= NST - 2:
                            # nothing reuses the nat pool after the final
                            # prep: these tiles stay resident for its context
                            kept_quads[st] = quad
                        for hc in range(NK):
                            tp = wps.tile([128, 512], MDT, tag="wp")
                            for p4 in range(4):
                                nc.tensor.transpose(
                                    tp[:, p4 * 128:(p4 + 1) * 128],
                                    quad[:, p4 * H + hc * 128:
                                         p4 * H + (hc + 1) * 128],
                                    ident[:])
                            vt = valtp.tile([128, 512], MDT, tag="vt")
                            if hc % 2 == 0:
                                nc.vector.tensor_copy(vt[:], tp[:])
                            else:
                                nc.scalar.copy(vt[:], tp[:])
                            valT[(st, hc)] = vt

                for stp in range(NST // 2):
                    load_and_transpose_pair(stp)
                    if stp == 0 and after_first_pair is not None:
                        after_first_pair()
                    sc_ps = []
                    for _i in range(2):
                        sc_t = sps.tile([1, 512], f32, tag="sc")
                        sc_ps.append(sc_t)
                    for ut in range(NK):
                        pps = []
                        for _i in range(2):
                            pp_t = wps.tile([128, 512], f32, tag="wp")
                            pps.append(pp_t)
                        for k in range(NK):
                            for st2 in range(2):
                                nc.tensor.matmul(
                                    pps[st2][:],
                                    w1u[ut][:, k * 128:(k + 1) * 128],
                                    valT[(stp * 2 + st2, k)][:],
                                    start=(k == 0), stop=(k == NK - 1))
                        if b == 0 and stp == 0:
                            # qbiasT[ut] = W2^T q + b1 + b2, squeezed into
                            # the pipeline just before its first use
                            qb_ps = wps.tile([128, BL], f32, tag="wp")
                            for k in range(NK):
                                nc.tensor.matmul(
                                    qb_ps[:], w2u[ut][:, k * 128:(k + 1) * 128],
                                    q_col[:, k * BL:(k + 1) * BL],
                                    start=(k == 0), stop=(k == NK - 1))
                            nc.vector.tensor_scalar_add(
                                qbT[ut][:], qb_ps[:], b12[:, ut:ut + 1])
                        for st2 in range(2):
                            th = tanhp.tile([128, 512], MDT, tag="th")
                            nc.scalar.activation(
                                th[:], pps[st2][:], AF.Tanh,
                                bias=qbT[ut][:, b:b + 1])
                            nc.tensor.matmul(
                                sc_ps[st2][:], v_col[:, ut:ut + 1], th[:],
                                start=(ut == 0), stop=(ut == NK - 1))
                    for st2 in range(2):
                        st = stp * 2 + st2
                        nc.vector.tensor_copy(
                            score_sb[:, st * 512:(st + 1) * 512],
                            sc_ps[st2][:])
                return score_sb

            def softmax_acol(b, score_sb):
                """softmax (in place on the score row) + column relayout."""
                m = rowp.tile([1, 1], f32, tag="m")
                nc.vector.reduce_max(m[:], score_sb[:], axis=AX.X)
                negm = rowp.tile([1, 1], f32, tag="negm")
                nc.vector.tensor_scalar_mul(negm[:], m[:], -1.0)
                esum = rowp.tile([1, 1], f32, tag="esum")
                nc.scalar.activation(score_sb[:], score_sb[:], AF.Exp,
                                     bias=negm[:], accum_out=esum[:])
                inv = rowp.tile([1, 1], f32, tag="inv")
                nc.vector.reciprocal(inv[:], esum[:])
                nc.vector.tensor_scalar_mul(score_sb[:], score_sb[:], inv[:])
                nc.sync.dma_start(att_o[b, :].rearrange("(o f) -> o f", o=1),
                                  score_sb[:])
                acps = sps.tile([128, S // 128], f32, tag="sc")
                for c in range(S // 128):
                    nc.tensor.matmul(
                        acps[:, c:c + 1],
                        score_sb[:, c * 128:(c + 1) * 128], ident1[:],
                        start=True, stop=True)
                acol = rowp.tile([128, S // 128], MDT, tag="acol")
                nc.vector.tensor_copy(acol[:], acps[:])
                return acol

            def cq_load(cnatp, b, pair):
                cq = cnatp.tile([128, 2 * H], MDT, tag="cnat")
                nc.gpsimd.dma_start(
                    cq[:].rearrange("p (p2 h) -> p p2 h", p2=2),
                    val[b, pair * 256:(pair + 1) * 256, :].rearrange(
                        "(p2 p) h -> p p2 h", p=128))
                return cq

            def ctx_batch(b, acol, cnatp, preloaded=()):
                """context = attn^T values, restreaming values from HBM."""
                crow = rowp.tile([1, H], f32, tag="crow")
                cp = []
                for _i in range(2):
                    cp_t = cps.tile([1, 512], f32, tag="cp")
                    cp.append(cp_t)
                cq = None
                preloaded = list(preloaded)
                resident = (kept_quads if b == BL - 1
                            else kept_quads2 if b == BL - 2 else {})
                for sc in range(S // 128):
                    st = sc // 4
                    if st in resident:
                        src = resident[st]
                        base = (sc % 4) * H
                    else:
                        if sc % 2 == 0:
                            if preloaded:
                                cq = preloaded.pop(0)
                            else:
                                cq = cq_load(cnatp, b, sc // 2)
                        src = cq
                        base = (sc % 2) * H
                    for half in range(2):
                        nc.tensor.matmul(
                            cp[half][:], acol[:, sc:sc + 1],
                            src[:, base + half * 512:base + half * 512 + 512],
                            start=(sc == 0), stop=(sc == S // 128 - 1))
                for half in range(2):
                    nc.vector.tensor_copy(
                        crow[:, half * 512:(half + 1) * 512], cp[half][:])
                nc.sync.dma_start(ctx_o[b, :].rearrange("(o f) -> o f", o=1),
                                  crow[:])

            # pipeline: A(b)=prep_scores, B(b)=softmax_acol, C(b)=ctx
            # order: A0 A1 B0 C0 A2 B1 C1 A3 B2 C2 B3 C3
            scores, acols = {}, {}
            scores[0] = prep_scores(0)
            w2ctx.__exit__(None, None, None)
            with tc.tile_pool(name="cpre", bufs=5) as cnatp, \
                 tc.tile_pool(name="keep", bufs=2) as keepp:
                pool_ref["keep"] = keepp
                for b in range(1, BL):
                    pre = []

                    def inject(bb=b - 1, dst=pre):
                        for p in range(4):
                            dst.append(cq_load(cnatp, bb, p))

                    scores[b] = prep_scores(b, after_first_pair=inject)
                    acols[b - 1] = softmax_acol(b - 1, scores.pop(b - 1))
                    ctx_batch(b - 1, acols.pop(b - 1), cnatp, pre)
                # prefetch part of the last batch's context quads so its
                # context matmuls aren't DMA-paced at the tail
                pre3 = [cq_load(cnatp, BL - 1, p) for p in range(4)]
                acols[BL - 1] = softmax_acol(BL - 1, scores.pop(BL - 1))
                ctx_batch(BL - 1, acols.pop(BL - 1), cnatp, pre3)

    nc.finalize()
    return nc


def _get_nc():
    key = MDT_NAME
    if key not in _cache:
        _cache[key] = _build(key)
    return _cache[key]


def kernel(query, values, W1, b1, W2, b2, V, bv):
    from concourse.bass_utils import run_bass_kernel_spmd

    query = np.ascontiguousarray(np.asarray(query, dtype=np.float32))
    values = np.ascontiguousarray(np.asarray(values, dtype=np.float32))
    W1 = np.ascontiguousarray(np.asarray(W1, dtype=np.float32))
    b1 = np.ascontiguousarray(np.asarray(b1, dtype=np.float32))
    W2 = np.ascontiguousarray(np.asarray(W2, dtype=np.float32))
    b2 = np.ascontiguousarray(np.asarray(b2, dtype=np.float32))
    V = np.ascontiguousarray(np.asarray(V, dtype=np.float32))
    bv = np.asarray(bv, dtype=np.float32)  # softmax-invariant; unused

    nc = _get_nc()
    in_maps = []
    for c in range(NCORES):
        lo = c * BL
        in_maps.append({
            "qry": query[lo:lo + BL],
            "val": values[lo:lo + BL],
            "W1": W1, "b1": b1, "W2": W2, "b2": b2, "Vw": V,
        })
    res = run_bass_kernel_spmd(nc, in_maps, list(range(NCORES)))
    ctx = np.concatenate([res.results[c]["ctx"] for c in range(NCORES)], axis=0)
    att = np.concatenate([res.results[c]["att"] for c in range(NCORES)], axis=0)
    return ctx, att.reshape(B, S, 1)


# revision 38
# speedup vs baseline: 1.0484x; 1.0026x over previous
"""Bahdanau attention TRN2 kernel.

B=32, S=2048, H=U=1024. Data-parallel over batch across 8 NeuronCores
(4 batches per core), weights replicated. All matmuls in fp32r
(full-rate fp32 with ~13-bit-mantissa rounding; rounding is done inline
by the SWDGE cast DMAs).

Per core, per batch:
  projT[u, s] = tanh( (W1^T @ values^T)[u, s] + (W2^T q + b1 + b2)[u] )
  score[s]    = V^T projT          (row layout [1, S])
  attn        = softmax(score)
  context[h]  = sum_s attn[s] * values[s, h]

values enters the PE twice: transposed (h on partitions, via PE
transpose) for proj, and natural (s on partitions, re-read from HBM)
for context. bv is dropped: softmax is shift-invariant.

Scheduling notes: per-engine execution order == emission order, so the
code is laid out as a software pipeline: weights stream ut-major so the
first proj starts as soon as values tile 0 + W1[:, :128] land; each
batch's context matmuls are emitted after the NEXT batch's score phase
so the PE never waits on a softmax.
"""
import sys
sys.path.insert(0, '/opt/trn_rl_repo')
import numpy as np

B, S, H, U = 32, 2048, 1024, 1024
NCORES = 8
BL = B // NCORES          # batches per core
NK = H // 128             # h/u 128-chunks
NST = S // 512            # s-tiles of 512
MDT_NAME = "float32r"     # matmul dtype

_cache = {}


def _build(mdt_name=MDT_NAME):
    from concourse import bacc, mybir, tile

    f32 = mybir.dt.float32
    MDT = getattr(mybir.dt, mdt_name)
    AF = mybir.ActivationFunctionType
    AX = mybir.AxisListType

    nc = bacc.Bacc()
    qry = nc.declare_dram_parameter("qry", [BL, H], f32, isOutput=False)
    val = nc.declare_dram_parameter("val", [BL, S, H], f32, isOutput=False)
    W1 = nc.declare_dram_parameter("W1", [H, U], f32, isOutput=False)
    b1 = nc.declare_dram_parameter("b1", [U], f32, isOutput=False)
    W2 = nc.declare_dram_parameter("W2", [H, U], f32, isOutput=False)
    b2 = nc.declare_dram_parameter("b2", [U], f32, isOutput=False)
    Vw = nc.declare_dram_parameter("Vw", [U, 1], f32, isOutput=False)
    ctx_o = nc.declare_dram_parameter("ctx", [BL, H], f32, isOutput=True)
    att_o = nc.declare_dram_parameter("att", [BL, S], f32, isOutput=True)

    ident_d = nc.inline_tensor(np.eye(128, dtype=np.float32), name="ident")

    def quad_load(dst, b, s_base):
        """One SWDGE cast DMA: values[b, s_base:s_base+512, :] ->
        dst[128, 4*H]; column block p4 holds s rows s_base+p4*128.."""
        nc.gpsimd.dma_start(
            dst[:].rearrange("p (p4 h) -> p p4 h", p4=4),
            val[b, s_base:s_base + 512, :].rearrange(
                "(p4 p) h -> p p4 h", p=128))

    def wu_load(dst, Wt, ut):
        """dst[128, NK*128] <- W[:, ut*128:(ut+1)*128] with h-chunk k at
        column block k: dst[p, k*128+u] = W[k*128+p, ut*128+u]."""
        nc.gpsimd.dma_start(
            dst[:].rearrange("p (k u) -> p k u", k=NK),
            Wt[:, ut * 128:(ut + 1) * 128].rearrange("(k p) u -> p k u", p=128))

    with tile.TileContext(nc) as tc:
        with tc.tile_pool(name="const", bufs=1) as cpool, \
             tc.tile_pool(name="nat", bufs=2) as natp, \
             tc.tile_pool(name="valT", bufs=20) as valtp, \
             tc.tile_pool(name="tanh", bufs=3) as tanhp, \
             tc.tile_pool(name="scorep", bufs=2) as scorep, \
             tc.tile_pool(name="rows", bufs=2) as rowp, \
             tc.tile_pool(name="wps", bufs=4, space="PSUM") as wps, \
             tc.tile_pool(name="sps", bufs=2, space="PSUM") as sps, \
             tc.tile_pool(name="cps", bufs=2, space="PSUM") as cps:

            # ---- DMA emission order is load order: get the critical path
            # (ident, first two value quads, W1/W2 col 0) in first ----
            ident = cpool.tile([128, 128], MDT, tag="ident")
            nc.gpsimd.dma_start(ident[:], ident_d[:])
            first_quads = {}
            for st in range(2):
                q_t = natp.tile([128, 4 * H], MDT, tag="nat")
                quad_load(q_t, 0, st * 512)
                first_quads[st] = q_t
            w2ctx = tc.tile_pool(name="w2p", bufs=1)
            w2pool = w2ctx.__enter__()
            w1u, w2u = [], []
            for ut in range(NK):
                w1_t = cpool.tile([128, NK * 128], MDT, tag=f"w1_{ut}")
                w1u.append(w1_t)
                w2_t = w2pool.tile([128, NK * 128], MDT, tag=f"w2_{ut}")
                w2u.append(w2_t)
            wu_load(w1u[0], W1, 0)
            wu_load(w2u[0], W2, 0)
            wu_load(w1u[1], W1, 1)
            # small constants
            v_col = cpool.tile([128, NK], MDT, tag="v_col")
            nc.gpsimd.dma_start(v_col[:], Vw[:, 0].rearrange("(k p) -> p k", p=128))
            q_col = cpool.tile([128, NK * BL], MDT, tag="q_col")
            for b in range(BL):
                nc.gpsimd.dma_start(
                    q_col[:].rearrange("p (k b) -> p k b", b=BL)[:, :, b],
                    qry[b, :].rearrange("(k p) -> p k", p=128))
            b12 = cpool.tile([128, NK], f32, tag="b12")
            b2c = cpool.tile([128, NK], f32, tag="b2c")
            nc.sync.dma_start(b12[:], b1[:].rearrange("(k p) -> p k", p=128))
            nc.sync.dma_start(b2c[:], b2[:].rearrange("(k p) -> p k", p=128))
            nc.vector.tensor_add(b12[:], b12[:], b2c[:])
            ident1 = cpool.tile([1, 1], f32, tag="ident1")
            nc.vector.memset(ident1[:], 1.0)
            # rest of the weights stream in behind the first compute
            wu_load(w2u[1], W2, 1)
            for ut in range(2, NK):
                wu_load(w1u[ut], W1, ut)
                wu_load(w2u[ut], W2, ut)

            qbT = []
            for u in range(NK):
                qb_t = cpool.tile([128, BL], f32, tag=f"qb_{u}")
                qbT.append(qb_t)

            kept_quads = {}
            kept_quads2 = {}
            pool_ref = {}

            def prep_scores(b, after_first_pair=None):
                """loads + transposes + proj/tanh/score for one batch."""
                valT = {}
                score_sb = scorep.tile([1, S], f32, tag="score")

                def load_and_transpose_pair(stp):
                    for st in (stp * 2, stp * 2 + 1):
                        if (b, st) in ((0, 0), (0, 1)):
                            quad = first_quads.pop(st)
                        elif b == BL - 2 and st >= NST - 2:
                            # batch BL-2's last pair stays resident (in the
                            # SBUF released by the W2 pool) so its context -
                            # which runs while batch BL-1 softmaxes - halves
                            # its HBM re-read
                            quad = pool_ref["keep"].tile(
                                [128, 4 * H], MDT, tag="keep")
                            quad_load(quad, b, st * 512)
                            kept_quads2[st] = quad
                        else:
                            quad = natp.tile([128, 4 * H], MDT, tag="nat")
                            quad_load(quad, b, st * 512)
                        if b == BL - 1 and st >= NST - 2:
                            # nothing reuses the nat pool after the final
                            # prep: these tiles stay resident for its context
                            kept_quads[st] = quad
                        for hc in range(NK):
                            tp = wps.tile([128, 512], MDT, tag="wp")
                            for p4 in range(4):
                                nc.tensor.transpose(
                                    tp[:, p4 * 128:(p4 + 1) * 128],
                                    quad[:, p4 * H + hc * 128:
                                         p4 * H + (hc + 1) * 128],
                                    ident[:])
                            vt = valtp.tile([128, 512], MDT, tag="vt")
                            if hc % 2 == 0:
                                nc.vector.tensor_copy(vt[:], tp[:])
                            else:
                                nc.scalar.copy(vt[:], tp[:])
                            valT[(st, hc)] = vt

                for stp in range(NST // 2):
                    load_and_transpose_pair(stp)
                    if stp == 0 and after_first_pair is not None:
                        after_first_pair()
                    sc_ps = []
                    for _i in range(2):
                        sc_t = sps.tile([1, 512], f32, tag="sc")
                        sc_ps.append(sc_t)
                    for ut in range(NK):
                        pps = []
                        for _i in range(2):
                            pp_t = wps.tile([128, 512], f32, tag="wp")
                            pps.append(pp_t)
                        for k in range(NK):
                            for st2 in range(2):
                                nc.tensor.matmul(
                                    pps[st2][:],
                                    w1u[ut][:, k * 128:(k + 1) * 128],
                                    valT[(stp * 2 + st2, k)][:],
                                    start=(k == 0), stop=(k == NK - 1))
                        if b == 0 and stp == 0:
                            # qbiasT[ut] = W2^T q + b1 + b2, squeezed into
                            # the pipeline just before its first use
                            qb_ps = wps.tile([128, BL], f32, tag="wp")
                            for k in range(NK):
                                nc.tensor.matmul(
                                    qb_ps[:], w2u[ut][:, k * 128:(k + 1) * 128],
                                    q_col[:, k * BL:(k + 1) * BL],
                                    start=(k == 0), stop=(k == NK - 1))
                            nc.vector.tensor_scalar_add(
                                qbT[ut][:], qb_ps[:], b12[:, ut:ut + 1])
                        for st2 in range(2):
                            th = tanhp.tile([128, 512], MDT, tag="th")
                            nc.scalar.activation(
                                th[:], pps[st2][:], AF.Tanh,
                                bias=qbT[ut][:, b:b + 1])
                            nc.tensor.matmul(
                                sc_ps[st2][:], v_col[:, ut:ut + 1], th[:],
                                start=(ut == 0), stop=(ut == NK - 1))
                    for st2 in range(2):
                        st = stp * 2 + st2
                        nc.vector.tensor_copy(
                            score_sb[:, st * 512:(st + 1) * 512],
                            sc_ps[st2][:])
                return score_sb

            def softmax_acol(b, score_sb):
                """softmax (in place on the score row) + column relayout."""
                m = rowp.tile([1, 1], f32, tag="m")
                nc.vector.reduce_max(m[:], score_sb[:], axis=AX.X)
                negm = rowp.tile([1, 1], f32, tag="negm")
                nc.vector.tensor_scalar_mul(negm[:], m[:], -1.0)
                esum = rowp.tile([1, 1], f32, tag="esum")
                nc.scalar.activation(score_sb[:], score_sb[:], AF.Exp,
                                     bias=negm[:], accum_out=esum[:])
                inv = rowp.tile([1, 1], f32, tag="inv")
                nc.vector.reciprocal(inv[:], esum[:])
                nc.vector.tensor_scalar_mul(score_sb[:], score_sb[:], inv[:])
                nc.sync.dma_start(att_o[b, :].rearrange("(o f) -> o f", o=1),
                                  score_sb[:])
                acps = sps.tile([128, S // 128], f32, tag="sc")
                for c in range(S // 128):
                    nc.tensor.matmul(
                        acps[:, c:c + 1],
                        score_sb[:, c * 128:(c + 1) * 128], ident1[:],
                        start=True, stop=True)
                acol = rowp.tile([128, S // 128], MDT, tag="acol")
                nc.vector.tensor_copy(acol[:], acps[:])
                return acol

            def cq_load(cnatp, b, pair):
                cq = cnatp.tile([128, 2 * H], MDT, tag="cnat")
                nc.gpsimd.dma_start(
                    cq[:].rearrange("p (p2 h) -> p p2 h", p2=2),
                    val[b, pair * 256:(pair + 1) * 256, :].rearrange(
                        "(p2 p) h -> p p2 h", p=128))
                return cq

            def ctx_batch(b, acol, cnatp, preloaded=()):
                """context = attn^T values, restreaming values from HBM."""
                crow = rowp.tile([1, H], f32, tag="crow")
                cp = []
                for _i in range(2):
                    cp_t = cps.tile([1, 512], f32, tag="cp")
                    cp.append(cp_t)
                cq = None
                preloaded = list(preloaded)
                resident = (kept_quads if b == BL - 1
                            else kept_quads2 if b == BL - 2 else {})
                for sc in range(S // 128):
                    st = sc // 4
                    if st in resident:
                        src = resident[st]
                        base = (sc % 4) * H
                    else:
                        if sc % 2 == 0:
                            if preloaded:
                                cq = preloaded.pop(0)
                            else:
                                cq = cq_load(cnatp, b, sc // 2)
                        src = cq
                        base = (sc % 2) * H
                    for half in range(2):
                        nc.tensor.matmul(
                            cp[half][:], acol[:, sc:sc + 1],
                            src[:, base + half * 512:base + half * 512 + 512],
                            start=(sc == 0), stop=(sc == S // 128 - 1))
                for half in range(2):
                    nc.vector.tensor_copy(
                        crow[:, half * 512:(half + 1) * 512], cp[half][:])
                nc.sync.dma_start(ctx_o[b, :].rearrange("(o f) -> o f", o=1),
                                  crow[:])

            # pipeline: A(b)=prep_scores, B(b)=softmax_acol, C(b)=ctx
            # order: A0 A1 B0 C0 A2 B1 C1 A3 B2 C2 B3 C3
            scores, acols = {}, {}
            scores[0] = prep_scores(0)
            w2ctx.__exit__(None, None, None)
            with tc.tile_pool(name="cpre", bufs=5) as cnatp, \
                 tc.tile_pool(name="keep", bufs=2) as keepp:
                pool_ref["keep"] = keepp
                for b in range(1, BL):
                    pre = []

                    def inject(bb=b - 1, dst=pre):
                        for p in range(4):
                            dst.append(cq_load(cnatp, bb, p))

                    scores[b] = prep_scores(b, after_first_pair=inject)
                    acols[b - 1] = softmax_acol(b - 1, scores.pop(b - 1))
                    ctx_batch(b - 1, acols.pop(b - 1), cnatp, pre)
                # prefetch part of the last batch's context quads so its
                # context matmuls aren't DMA-paced at the tail
                pre3 = [cq_load(cnatp, BL - 1, p) for p in range(4)]
                acols[BL - 1] = softmax_acol(BL - 1, scores.pop(BL - 1))
                ctx_batch(BL - 1, acols.pop(BL - 1), cnatp, pre3)

    nc.finalize()
    return nc


def _get_nc():
    key = MDT_NAME
    if key not in _cache:
        _cache[key] = _build(key)
    return _cache[key]


def kernel(query, values, W1, b1, W2, b2, V, bv):
    from concourse.bass_utils import run_bass_kernel_spmd

    query = np.ascontiguousarray(np.asarray(query, dtype=np.float32))
    values = np.ascontiguousarray(np.asarray(values, dtype=np.float32))
    W1 = np.ascontiguousarray(np.asarray(W1, dtype=np.float32))
    b1 = np.ascontiguousarray(np.asarray(b1, dtype=np.float32))
    W2 = np.ascontiguousarray(np.asarray(W2, dtype=np.float32))
    b2 = np.ascontiguousarray(np.asarray(b2, dtype=np.float32))
    V = np.ascontiguousarray(np.asarray(V, dtype=np.float32))
    bv = np.asarray(bv, dtype=np.float32)  # softmax-invariant; unused

    nc = _get_nc()
    in_maps = []
    for c in range(NCORES):
        lo = c * BL
        in_maps.append({
            "qry": query[lo:lo + BL],
            "val": values[lo:lo + BL],
            "W1": W1, "b1": b1, "W2": W2, "b2": b2, "Vw": V,
        })
    res = run_bass_kernel_spmd(nc, in_maps, list(range(NCORES)))
    ctx = np.concatenate([res.results[c]["ctx"] for c in range(NCORES)], axis=0)
    att = np.concatenate([res.results[c]["att"] for c in range(NCORES)], axis=0)
    return ctx, att.reshape(B, S, 1)
